# revision 39
# baseline (speedup 1.0000x reference)
"""Bass/Trainium2 kernel for GQA transformer block (nn_GQA_84353157694016).

Reference computation (B=2, S=2048, E=4096, H=32 q-heads, KVH=8 kv-heads, D=128):
    qkv = x @ wqkv.T                  -> split into q/k/v per GQA group
    q,k = rope_interleaved(q), rope_interleaved(k)
    out = softmax(causal(q k^T / sqrt(D))) @ v @ wo.T

Sharding (8 cores): data-parallel over batch (2 groups of 4 cores) x
tensor-parallel over heads (4 cores: 2 kv groups / 8 q heads each).  wo is
sharded on its input dim; the partial outputs are summed on the host
(the unshard step of the reduce).

Layout strategy: everything on-chip is computed in "transposed" (feature x
sequence) orientation so the TensorE contraction dim always lands on
partitions with zero on-chip transposes (v is transposed via the DMA XBAR).
Softmax is computed without max-subtraction (scores are bounded ~ +-10 for
this problem size/scale), with the row-sum obtained by an extra ones-matmul
in the same PSUM-accumulation pass as attn @ v.

The q/k feature dims are de-interleaved host-side (wqkv row permutation) so
RoPE's pair-swap becomes two half-partition adds on the vector engine
instead of strided SBUF-to-SBUF DMAs.  QK^T dot products are invariant to
the (shared) feature permutation; v rows are left in original order.
"""

import os
import sys

import numpy as np
import ml_dtypes

for _p in ("/opt/trn_rl_repo",):
    if _p not in sys.path and os.path.isdir(_p):
        sys.path.append(_p)

import concourse.bass as bass
import concourse.tile as tile
from concourse import bacc, mybir
from concourse.bass_utils import run_bass_kernel_spmd
from concourse.masks import make_identity


def _install_ntff_hook():
    """bass_utils' trace path imports antenv.axon_hooks, which the agent image
    lacks; synthesize it (backed by trn_boot's ctypes NTFF driver) so
    trace=True / BASS_TRACE=1 works instead of crashing."""
    try:
        import antenv.axon_hooks  # noqa: F401
        return
    except ImportError:
        pass
    try:
        import types
        import antenv
        mod = types.ModuleType("antenv.axon_hooks")
        mod._hook = None
        mod.set_axon_ntff_profile_hook = lambda h: setattr(mod, "_hook", h)
        mod.get_axon_ntff_profile_hook = lambda: mod._hook
        sys.modules["antenv.axon_hooks"] = mod
        antenv.axon_hooks = mod
        from trn_agent_boot.trn_boot import _ntff_profile_via_ctypes
        so = "/opt/axon/libaxon_pjrt.so"
        if os.path.exists(so):
            mod._hook = _ntff_profile_via_ctypes(so)
    except Exception:
        pass


_install_ntff_hook()

# problem constants
B, S, E = 2, 2048, 4096
H, KVH, D = 32, 8, 128
QPK = H // KVH                    # 4 q heads per kv group
ROPE_BASE = 10000.0

NCORES = 8
TP = 4                            # tensor-parallel width (heads)
DP = 2                            # data-parallel width (batch)

SC = 4                            # number of s-chunks == q strips
CW = S // SC                      # 512 chunk width
NJT = (E + 2 * KVH * D) // TP // 128   # 12 qkv row-tiles per core
NET = E // 128                    # 32 contraction tiles for qkv proj
GPC = KVH // TP                   # 2 kv groups per core
HPC = H // TP                     # 8 q heads per core
FT = HPC * D // 128               # 8 local ctx feature tiles
ECN = E // CW                     # 8 output e-chunks
SB = 16                           # row-sum ones-matmul batch size

f32 = mybir.dt.float32
bf16 = mybir.dt.bfloat16
np_bf16 = ml_dtypes.bfloat16

_built = {}


def _build_nc():
    nc = bacc.Bacc("TRN2", target_bir_lowering=False)

    xt_d = nc.dram_tensor("xt", [SC, 128, NET, CW], bf16, kind="ExternalInput")
    wq_d = nc.dram_tensor("wq", [NJT, 128, NET, 128], bf16, kind="ExternalInput")
    wo_d = nc.dram_tensor("wo", [ECN, 128, FT, CW], bf16, kind="ExternalInput")
    cq_d = nc.dram_tensor("cq", [128, S], f32, kind="ExternalInput")
    sq_d = nc.dram_tensor("sq", [128, S], f32, kind="ExternalInput")
    ck_d = nc.dram_tensor("ck", [128, S], f32, kind="ExternalInput")
    sk_d = nc.dram_tensor("sk", [128, S], f32, kind="ExternalInput")
    mk_d = nc.dram_tensor("mk", [128, 128], bf16, kind="ExternalInput")
    out_d = nc.dram_tensor("out", [S // 128, ECN, 128, CW], bf16,
                           kind="ExternalOutput")

    with tile.TileContext(nc) as tc:
        with (
            tc.tile_pool(name="const", bufs=1) as constp,
            tc.tile_pool(name="tab", bufs=1) as tabp,
            tc.tile_pool(name="xt", bufs=3) as xtp,
            tc.tile_pool(name="wq", bufs=6) as wqp,
            tc.tile_pool(name="st", bufs=7) as stp,
            tc.tile_pool(name="rt", bufs=2) as rtp,
            tc.tile_pool(name="q", bufs=1) as qp,
            tc.tile_pool(name="kv", bufs=1) as kvp,
            tc.tile_pool(name="at", bufs=8) as atp,
            tc.tile_pool(name="ctx", bufs=2) as ctxp,
            tc.tile_pool(name="wop", bufs=3) as wop,
            tc.tile_pool(name="ob", bufs=3) as obp,
            tc.tile_pool(name="rc", bufs=1) as rcp,
            tc.tile_pool(name="pmm", bufs=2, space="PSUM") as pmm,
            tc.tile_pool(name="pqk", bufs=3, space="PSUM") as pqk,
            tc.tile_pool(name="pacc", bufs=3, space="PSUM") as pacc,
        ):
            def load_wo(ec, drain=False):
                """Start the wo tile load for output chunk ec.  Halves split
                across sync (clear of weight prefetch during the attention
                phase) and gpsimd; never scalar, whose FIFO carries the
                critical EXPs -- except in the drain, where there are no
                EXPs left and gpsimd (SW DGE) is the laggard."""
                wo_sb = wop.tile([128, FT, CW], bf16, tag="wo", name="wo_sb")
                eng2 = nc.scalar if drain else nc.gpsimd
                nc.sync.dma_start(out=wo_sb[:, :FT // 2, :],
                                  in_=wo_d[ec, :, :FT // 2, :])
                eng2.dma_start(out=wo_sb[:, FT // 2:, :],
                               in_=wo_d[ec, :, FT // 2:, :])
                return wo_sb

            def emit_wo_block(cs, ec, ctx_tiles, wo_sb, drain=False):
                """Output-projection block: out[strip cs, ec] += ctx @ woT."""
                eng2 = nc.scalar if drain else nc.gpsimd
                for sti in range(CW // 128):
                    ps = pacc.tile([128, CW], f32, tag="acc", name="wo_ps")
                    for ft in range(FT):
                        nc.tensor.matmul(
                            ps,
                            lhsT=ctx_tiles[:, ft, sti * 128:(sti + 1) * 128],
                            rhs=wo_sb[:, ft, :],
                            start=(ft == 0),
                            stop=(ft == FT - 1),
                        )
                    ob = obp.tile([128, CW], bf16, tag="ob", name="ob")
                    nc.vector.tensor_copy(ob, ps)
                    eng2.dma_start(
                        out=out_d[(CW // 128) * cs + sti, ec], in_=ob
                    )

            # constants
            ident = constp.tile([128, 128], f32, tag="ident")
            make_identity(nc, ident)
            ones_sb = constp.tile([128, 128], bf16, tag="ones")
            nc.vector.memset(ones_sb, 1.0)
            mk_sb = constp.tile([128, 128], bf16, tag="mk")

            # persistent k (transposed) and v (natural) per kv group, bf16
            k_sb = [kvp.tile([128, S], bf16, tag=f"k{g}", name=f"k{g}")
                    for g in range(GPC)]
            v_sb = [kvp.tile([128, S // 128, 128], bf16, tag=f"v{g}", name=f"v{g}")
                    for g in range(GPC)]

            def load_tables(c):
                """Rope table slices for strip c (needed only at RoPE time)."""
                csl = slice(c * CW, (c + 1) * CW)
                cq_sb = tabp.tile([128, CW], f32, tag="cq")
                sq_sb = tabp.tile([128, CW], f32, tag="sq")
                ck_sb = tabp.tile([128, CW], f32, tag="ck")
                sk_sb = tabp.tile([128, CW], f32, tag="sk")
                nc.sync.dma_start(out=cq_sb, in_=cq_d[:, csl])
                nc.sync.dma_start(out=sq_sb, in_=sq_d[:, csl])
                nc.sync.dma_start(out=ck_sb, in_=ck_d[:, csl])
                nc.sync.dma_start(out=sk_sb, in_=sk_d[:, csl])
                return cq_sb, sq_sb, ck_sb, sk_sb

            nxt = {}
            for c in range(SC):
                csl = slice(c * CW, (c + 1) * CW)
                # Weight loads for the whole strip, issued up-front in
                # consumption order on the SYNC queue only: a WAR-delayed
                # prefetch there can only delay later prefetches, never the
                # scalar queue's EXPs/copies or the gpsimd queue's work.
                # x/tables for strips >0 were hoisted into the previous
                # strip's attention phase.  Strip 0's leading tiles are
                # split into pieces across sync+scalar (those DMAs never
                # wait) for a fast start.
                wq_tiles = []
                if c > 0:
                    xt_h = nxt.pop("x")
                    tabs = nxt.pop("tabs")
                else:
                    xt_h = []
                for jt in range(NJT):
                    w_ = wqp.tile([128, NET, 128], bf16, tag="wq", name="wq_sb")
                    if c == 0 and jt < 2:
                        # interleave the first weight tiles and x halves in
                        # small pieces across both HW queues; geometrically
                        # growing pieces so the very first matmuls can start
                        # after ~0.4 MB has landed
                        xh = xtp.tile([128, NET // 2, CW], bf16, tag="xt",
                                      name="xh")
                        cuts = (0, 4, 8, 16, 32) if jt == 0 else (0, 16, 32)
                        for pi in range(len(cuts) - 1):
                            w0, w1 = cuts[pi], cuts[pi + 1]
                            e0, e1 = w0 // 2, w1 // 2
                            nc.sync.dma_start(out=w_[:, w0:w1, :],
                                              in_=wq_d[jt, :, w0:w1, :])
                            nc.scalar.dma_start(
                                out=xh[:, e0:e1, :],
                                in_=xt_d[c, :, jt * (NET // 2) + e0:
                                         jt * (NET // 2) + e1, :])
                        xt_h.append(xh)
                    else:
                        nc.sync.dma_start(out=w_, in_=wq_d[jt])
                    wq_tiles.append(w_)
                    if c == 0 and jt == 3:
                        tabs = load_tables(c)
                if c == 0:
                    nc.sync.dma_start(out=mk_sb, in_=mk_d[:])
                cq_sb, sq_sb, ck_sb, sk_sb = tabs

                # ---- fused QKV projection + RoPE + v transpose, per kv group ----
                q_sb = qp.tile([128, HPC, CW], bf16, tag="q")
                for g in range(GPC):
                    stage = []
                    for sub in range(6):     # 4 q tiles, 1 k tile, 1 v tile
                        jt = 6 * g + sub
                        wq_sb = wq_tiles[jt]
                        ps = pmm.tile([128, CW], f32, tag="mm")
                        for et in range(NET):
                            nc.tensor.matmul(
                                ps,
                                lhsT=wq_sb[:, et, :],
                                rhs=xt_h[et // (NET // 2)][:, et % (NET // 2), :],
                                start=(et == 0),
                                stop=(et == NET - 1),
                            )
                        st = stp.tile([128, CW], f32, tag="st")
                        nc.scalar.copy(st, ps)
                        stage.append(st)
                    # RoPE on 4 q tiles + 1 k tile; features are
                    # de-interleaved (pair halves in partitions 0-63/64-127)
                    # so the pair-swap is two contiguous half-partition DMAs
                    for sub in range(QPK + 1):
                        stq = stage[sub]
                        is_q = sub < QPK
                        c_tab = cq_sb if is_q else ck_sb
                        s_tab = sq_sb if is_q else sk_sb
                        sw = rtp.tile([128, CW], f32, tag="sw")
                        nc.gpsimd.dma_start(out=sw[0:64, :], in_=stq[64:128, :])
                        nc.gpsimd.dma_start(out=sw[64:128, :], in_=stq[0:64, :])
                        xs = rtp.tile([128, CW], f32, tag="xs")
                        nc.vector.tensor_mul(xs, sw, s_tab)
                        nc.vector.tensor_mul(stq, stq, c_tab)
                        if is_q:
                            nc.vector.tensor_add(q_sb[:, QPK * g + sub, :],
                                                 stq, xs)
                        else:
                            nc.vector.tensor_add(k_sb[g][:, csl], stq, xs)
                    # v transpose on the PE (never touches a DMA queue); uses
                    # the qk PSUM pool, which is idle during the projection
                    # phase, so it never contends with the matmul chains
                    stv = stage[5]
                    for u in range(CW // 128):
                        tp_ = pqk.tile([128, CW], f32, tag="qk")
                        nc.tensor.transpose(
                            tp_[:, :128], stv[:, u * 128:(u + 1) * 128], ident
                        )
                        nc.scalar.copy(
                            v_sb[g][:, (CW // 128) * c + u, :], tp_[:, :128]
                        )

                # ---- attention for q strip c (flash-style, no max) ----
                njt2 = (CW // 128) * (c + 1)     # causal: k tiles 0..4c+3
                ctx_sb = ctxp.tile([128, HPC, CW], bf16, tag="ctx")
                if c > 0:
                    wo_q = [load_wo(0), load_wo(1)]
                for g in range(GPC):
                    for hq in range(QPK):
                        h = QPK * g + hq
                        if c > 0:
                            # software pipeline: strip c-1's output projection
                            # block (ec = h) fills PE while ACT/DVE run
                            # softmax; block weights prefetch two head-blocks
                            # ahead
                            if h < ECN - 2:
                                wo_q.append(load_wo(h + 2))
                            elif c == SC - 1:
                                # pre-load the drain's first blocks while the
                                # queues are quiet (no next-strip prefetch)
                                wo_q.append(load_wo(h - ECN + 2, drain=True))
                            emit_wo_block(c - 1, h, prev_ctx, wo_q.pop(0))
                        ctx_ps = pacc.tile([128, CW], f32, tag="acc")
                        sums_ps = pacc.tile([128, CW], f32, tag="acc")
                        at_acc = None
                        nsg = (njt2 + SB - 1) // SB   # ones-matmul groups
                        for j2 in range(njt2):
                            # diagonal k-tiles: trim the fully-masked columns
                            # from the QK matmul, exp and AV; zero-fill that
                            # part of the attn tile so the row-sums stay
                            # full-width
                            diag = j2 >= njt2 - (CW // 128)
                            o = 128 * (j2 - (njt2 - (CW // 128))) if diag else 0
                            nw = CW - o
                            qk = pqk.tile([128, CW], f32, tag="qk")
                            nc.tensor.matmul(
                                qk[:, :nw],
                                lhsT=k_sb[g][:, j2 * 128:(j2 + 1) * 128],
                                rhs=q_sb[:, h, o:],
                                start=True, stop=True,
                            )
                            at = atp.tile([128, CW], bf16, tag="at")
                            if o:
                                nc.gpsimd.memset(at[:, :o], 0.0)
                            nc.scalar.activation(
                                at[:, o:], qk[:, :nw],
                                mybir.ActivationFunctionType.Exp
                            )
                            if diag:
                                nc.vector.tensor_mul(
                                    at[:, o:o + 128], at[:, o:o + 128],
                                    mk_sb,
                                )
                            first, last = j2 == 0, j2 == njt2 - 1
                            if o:
                                nc.tensor.matmul(
                                    ctx_ps[:, o:], lhsT=v_sb[g][:, j2, :],
                                    rhs=at[:, o:], start=False, stop=last,
                                )
                            else:
                                nc.tensor.matmul(
                                    ctx_ps, lhsT=v_sb[g][:, j2, :], rhs=at,
                                    start=first, stop=last,
                                )
                            # batch the row-sum matmul over groups of SB attn
                            # tiles: accumulate on DVE (bf16), one ones-matmul
                            # per group instead of per tile
                            ph = j2 % SB
                            if ph == 0:
                                at_prev = at
                            elif ph == 1:
                                at_acc = atp.tile([128, CW], bf16, tag="ata",
                                                  name="at_acc", bufs=3)
                                nc.vector.tensor_add(at_acc, at_prev, at)
                            else:
                                nc.vector.tensor_add(at_acc, at_acc, at)
                            if ph == SB - 1 or j2 == njt2 - 1:
                                grp = j2 // SB
                                src = at_acc if ph else at_prev
                                nc.tensor.matmul(
                                    sums_ps, lhsT=ones_sb, rhs=src,
                                    start=(grp == 0), stop=(grp == nsg - 1),
                                )
                        rc = rcp.tile([128, CW], f32, tag="rc")
                        nc.vector.reciprocal_approx_fast(out=rc, in_=sums_ps)
                        nc.vector.tensor_mul(ctx_sb[:, h, :], ctx_ps, rc)

                        # hoist the next strip's x/table loads into this
                        # attention phase, one piece per head so the wo
                        # loads interleaved above are never delayed by more
                        # than ~1 MB of queue occupancy
                        if c < SC - 1:
                            if h == 0:
                                nxt["x"] = [
                                    xtp.tile([128, NET // 2, CW], bf16,
                                             tag="xt", name="xh")
                                    for _ in range(2)
                                ]
                            if h < 4:
                                s0, s1 = (h % 2) * (NET // 4), \
                                    (h % 2 + 1) * (NET // 4)
                                nc.sync.dma_start(
                                    out=nxt["x"][h // 2][:, s0:s1, :],
                                    in_=xt_d[c + 1, :,
                                             (h // 2) * (NET // 2) + s0:
                                             (h // 2) * (NET // 2) + s1, :])
                            elif h == 4:
                                nxt["tabs"] = load_tables(c + 1)

                prev_ctx = ctx_sb

            # drain: output projection for the final strip (blocks 0 and 1
            # were pre-loaded during the last attention phase)
            for ec in range(ECN):
                if ec < ECN - 2:
                    wo_q.append(load_wo(ec + 2, drain=True))
                emit_wo_block(SC - 1, ec, prev_ctx, wo_q.pop(0), drain=True)
    nc.finalize()
    return nc


def _rope_tables(scale):
    """De-interleaved rope tables [128, S]: partition p<64 holds pair-lane 1
    (original feature 2p), p>=64 holds pair-lane 2 (feature 2(p-64)+1).
    The S table multiplies the half-SWAPPED tensor (sw[p<64] = x2):
      out_top = x1*cos + sw_top * (-sin) = x1 cos - x2 sin
      out_bot = x2*cos + sw_bot * (+sin) = x2 cos + x1 sin
    """
    inv = 1.0 / (ROPE_BASE ** (np.arange(0, D, 2, dtype=np.float64) / D))
    ang = np.arange(S, dtype=np.float64)[None, :] * inv[:, None]    # [D/2, S]
    C = np.empty((D, S), np.float32)
    Sx = np.empty((D, S), np.float32)
    C[:64] = np.cos(ang)
    C[64:] = np.cos(ang)
    Sx[:64] = -np.sin(ang)
    Sx[64:] = np.sin(ang)
    return (C * scale).astype(np.float32), (Sx * scale).astype(np.float32)


_DEPERM = np.concatenate([np.arange(0, D, 2), np.arange(1, D, 2)])


def _host_inputs(x, wqkv, wo):
    """Shard + retile inputs for the 8 cores. Core c = 4*db + t."""
    cq, sq = _rope_tables(D ** -0.5)
    ck, sk = _rope_tables(1.0)

    # causal mask for the 128x128 diagonal block, scores^T layout [k, q]
    jj = np.arange(128)[:, None]
    ii = np.arange(128)[None, :]
    mk = (jj <= ii).astype(np_bf16)

    # de-interleave the q/k head feature rows of wqkv (see _rope_tables);
    # v rows keep their original order
    wq_p = wqkv.reshape(KVH, QPK + 2, D, E).copy()
    wq_p[:, :QPK + 1] = wq_p[:, :QPK + 1, _DEPERM]
    wq_p = wq_p.reshape(E + 2 * KVH * D, E)

    xts = []
    for db in range(DP):
        xT = np.ascontiguousarray(x[db].T)                 # [E, S]
        t = xT.reshape(NET, 128, SC, CW).transpose(2, 1, 0, 3)
        xts.append(np.ascontiguousarray(t.astype(np_bf16)))

    wqs, wos = [], []
    rows = (E + 2 * KVH * D) // TP
    for t in range(TP):
        wT = np.ascontiguousarray(wq_p[rows * t:rows * (t + 1)].T)   # [E, 1536]
        wq_t = wT.reshape(NET, 128, NJT, 128).transpose(2, 1, 0, 3)
        wqs.append(np.ascontiguousarray(wq_t.astype(np_bf16)))
        woT = np.ascontiguousarray(wo[:, 1024 * t:1024 * (t + 1)].T)  # [1024, E]
        wo_t = woT.reshape(FT, 128, ECN, CW).transpose(2, 1, 0, 3)
        wos.append(np.ascontiguousarray(wo_t.astype(np_bf16)))

    in_maps = []
    for c in range(NCORES):
        db, t = divmod(c, TP)
        in_maps.append({
            "xt": xts[db], "wq": wqs[t], "wo": wos[t],
            "cq": cq, "sq": sq, "ck": ck, "sk": sk,
            "mk": mk,
        })
    return in_maps


def kernel(x, wqkv, wo):
    x = np.asarray(x, np.float32)
    wqkv = np.asarray(wqkv, np.float32)
    wo = np.asarray(wo, np.float32)

    if "nc" not in _built:
        _built["nc"] = _build_nc()
    nc = _built["nc"]

    if os.environ.get("BASS_TRACE") and "warm" not in _built:
        # axon's NTFF profile start returns -1 until the PJRT client has
        # fully initialized (first execute), so force one tiny op first.
        try:
            import jax
            jax.jit(lambda a: a + 1)(np.zeros(1, np.float32))
        except Exception:
            pass
        _built["warm"] = True

    in_maps = _host_inputs(x, wqkv, wo)
    res = run_bass_kernel_spmd(nc, in_maps, core_ids=list(range(NCORES)))
    globals()["_last_results"] = res

    out = np.zeros((B, S, E), np.float32)
    for c in range(NCORES):
        db = c // TP
        o = res.results[c]["out"]                       # [16, 8, 128, 512] bf16
        out[db] += o.astype(np.float32).transpose(0, 2, 1, 3).reshape(S, E)
    return out


# revision 40
# speedup vs baseline: 1.0043x; 1.0043x over previous
"""Bass/Trainium2 kernel for GQA transformer block (nn_GQA_84353157694016).

Reference computation (B=2, S=2048, E=4096, H=32 q-heads, KVH=8 kv-heads, D=128):
    qkv = x @ wqkv.T                  -> split into q/k/v per GQA group
    q,k = rope_interleaved(q), rope_interleaved(k)
    out = softmax(causal(q k^T / sqrt(D))) @ v @ wo.T

Sharding (8 cores): data-parallel over batch (2 groups of 4 cores) x
tensor-parallel over heads (4 cores: 2 kv groups / 8 q heads each).  wo is
sharded on its input dim; the partial outputs are summed on the host
(the unshard step of the reduce).

Layout strategy: everything on-chip is computed in "transposed" (feature x
sequence) orientation so the TensorE contraction dim always lands on
partitions with zero on-chip transposes (v is transposed via the DMA XBAR).
Softmax is computed without max-subtraction (scores are bounded ~ +-10 for
this problem size/scale), with the row-sum obtained by an extra ones-matmul
in the same PSUM-accumulation pass as attn @ v.

The q/k feature dims are de-interleaved host-side (wqkv row permutation) so
RoPE's pair-swap becomes two half-partition adds on the vector engine
instead of strided SBUF-to-SBUF DMAs.  QK^T dot products are invariant to
the (shared) feature permutation; v rows are left in original order.
"""

import os
import sys

import numpy as np
import ml_dtypes

for _p in ("/opt/trn_rl_repo",):
    if _p not in sys.path and os.path.isdir(_p):
        sys.path.append(_p)

import concourse.bass as bass
import concourse.tile as tile
from concourse import bacc, mybir
from concourse.bass_utils import run_bass_kernel_spmd
from concourse.masks import make_identity


def _install_ntff_hook():
    """bass_utils' trace path imports antenv.axon_hooks, which the agent image
    lacks; synthesize it (backed by trn_boot's ctypes NTFF driver) so
    trace=True / BASS_TRACE=1 works instead of crashing."""
    try:
        import antenv.axon_hooks  # noqa: F401
        return
    except ImportError:
        pass
    try:
        import types
        import antenv
        mod = types.ModuleType("antenv.axon_hooks")
        mod._hook = None
        mod.set_axon_ntff_profile_hook = lambda h: setattr(mod, "_hook", h)
        mod.get_axon_ntff_profile_hook = lambda: mod._hook
        sys.modules["antenv.axon_hooks"] = mod
        antenv.axon_hooks = mod
        from trn_agent_boot.trn_boot import _ntff_profile_via_ctypes
        so = "/opt/axon/libaxon_pjrt.so"
        if os.path.exists(so):
            mod._hook = _ntff_profile_via_ctypes(so)
    except Exception:
        pass


_install_ntff_hook()

# problem constants
B, S, E = 2, 2048, 4096
H, KVH, D = 32, 8, 128
QPK = H // KVH                    # 4 q heads per kv group
ROPE_BASE = 10000.0

NCORES = 8
TP = 4                            # tensor-parallel width (heads)
DP = 2                            # data-parallel width (batch)

SC = 4                            # number of s-chunks == q strips
CW = S // SC                      # 512 chunk width
NJT = (E + 2 * KVH * D) // TP // 128   # 12 qkv row-tiles per core
NET = E // 128                    # 32 contraction tiles for qkv proj
GPC = KVH // TP                   # 2 kv groups per core
HPC = H // TP                     # 8 q heads per core
FT = HPC * D // 128               # 8 local ctx feature tiles
ECN = E // CW                     # 8 output e-chunks
SB = 8                            # row-sum ones-matmul batch size

f32 = mybir.dt.float32
bf16 = mybir.dt.bfloat16
np_bf16 = ml_dtypes.bfloat16

_built = {}


def _build_nc():
    nc = bacc.Bacc("TRN2", target_bir_lowering=False)

    xt_d = nc.dram_tensor("xt", [SC, 128, NET, CW], bf16, kind="ExternalInput")
    wq_d = nc.dram_tensor("wq", [NJT, 128, NET, 128], bf16, kind="ExternalInput")
    wo_d = nc.dram_tensor("wo", [ECN, 128, FT, CW], bf16, kind="ExternalInput")
    cq_d = nc.dram_tensor("cq", [128, S], f32, kind="ExternalInput")
    sq_d = nc.dram_tensor("sq", [128, S], f32, kind="ExternalInput")
    ck_d = nc.dram_tensor("ck", [128, S], f32, kind="ExternalInput")
    sk_d = nc.dram_tensor("sk", [128, S], f32, kind="ExternalInput")
    mk_d = nc.dram_tensor("mk", [128, 128], bf16, kind="ExternalInput")
    out_d = nc.dram_tensor("out", [S // 128, ECN, 128, CW], bf16,
                           kind="ExternalOutput")

    with tile.TileContext(nc) as tc:
        with (
            tc.tile_pool(name="const", bufs=1) as constp,
            tc.tile_pool(name="tab", bufs=1) as tabp,
            tc.tile_pool(name="xt", bufs=3) as xtp,
            tc.tile_pool(name="wq", bufs=6) as wqp,
            tc.tile_pool(name="st", bufs=7) as stp,
            tc.tile_pool(name="rt", bufs=2) as rtp,
            tc.tile_pool(name="q", bufs=1) as qp,
            tc.tile_pool(name="kv", bufs=1) as kvp,
            tc.tile_pool(name="at", bufs=8) as atp,
            tc.tile_pool(name="ctx", bufs=2) as ctxp,
            tc.tile_pool(name="wop", bufs=3) as wop,
            tc.tile_pool(name="ob", bufs=3) as obp,
            tc.tile_pool(name="rc", bufs=1) as rcp,
            tc.tile_pool(name="pmm", bufs=2, space="PSUM") as pmm,
            tc.tile_pool(name="pqk", bufs=3, space="PSUM") as pqk,
            tc.tile_pool(name="pacc", bufs=3, space="PSUM") as pacc,
        ):
            def load_wo(ec, drain=False):
                """Start the wo tile load for output chunk ec.  Halves split
                across sync (clear of weight prefetch during the attention
                phase) and gpsimd; never scalar, whose FIFO carries the
                critical EXPs -- except in the drain, where there are no
                EXPs left and gpsimd (SW DGE) is the laggard."""
                wo_sb = wop.tile([128, FT, CW], bf16, tag="wo", name="wo_sb")
                eng2 = nc.scalar if drain else nc.gpsimd
                nc.sync.dma_start(out=wo_sb[:, :FT // 2, :],
                                  in_=wo_d[ec, :, :FT // 2, :])
                eng2.dma_start(out=wo_sb[:, FT // 2:, :],
                               in_=wo_d[ec, :, FT // 2:, :])
                return wo_sb

            def emit_wo_block(cs, ec, ctx_tiles, wo_sb, drain=False):
                """Output-projection block: out[strip cs, ec] += ctx @ woT."""
                eng2 = nc.scalar if drain else nc.gpsimd
                for sti in range(CW // 128):
                    ps = pacc.tile([128, CW], f32, tag="acc", name="wo_ps")
                    for ft in range(FT):
                        nc.tensor.matmul(
                            ps,
                            lhsT=ctx_tiles[:, ft, sti * 128:(sti + 1) * 128],
                            rhs=wo_sb[:, ft, :],
                            start=(ft == 0),
                            stop=(ft == FT - 1),
                        )
                    ob = obp.tile([128, CW], bf16, tag="ob", name="ob")
                    nc.vector.tensor_copy(ob, ps)
                    eng2.dma_start(
                        out=out_d[(CW // 128) * cs + sti, ec], in_=ob
                    )

            # constants
            ident = constp.tile([128, 128], f32, tag="ident")
            make_identity(nc, ident)
            ones_sb = constp.tile([128, 128], bf16, tag="ones")
            nc.vector.memset(ones_sb, 1.0)
            mk_sb = constp.tile([128, 128], bf16, tag="mk")

            # persistent k (transposed) and v (natural) per kv group, bf16
            k_sb = [kvp.tile([128, S], bf16, tag=f"k{g}", name=f"k{g}")
                    for g in range(GPC)]
            v_sb = [kvp.tile([128, S // 128, 128], bf16, tag=f"v{g}", name=f"v{g}")
                    for g in range(GPC)]

            def load_tables(c):
                """Rope table slices for strip c (needed only at RoPE time)."""
                csl = slice(c * CW, (c + 1) * CW)
                cq_sb = tabp.tile([128, CW], f32, tag="cq")
                sq_sb = tabp.tile([128, CW], f32, tag="sq")
                ck_sb = tabp.tile([128, CW], f32, tag="ck")
                sk_sb = tabp.tile([128, CW], f32, tag="sk")
                nc.sync.dma_start(out=cq_sb, in_=cq_d[:, csl])
                nc.sync.dma_start(out=sq_sb, in_=sq_d[:, csl])
                nc.sync.dma_start(out=ck_sb, in_=ck_d[:, csl])
                nc.sync.dma_start(out=sk_sb, in_=sk_d[:, csl])
                return cq_sb, sq_sb, ck_sb, sk_sb

            nxt = {}
            for c in range(SC):
                csl = slice(c * CW, (c + 1) * CW)
                # Weight loads for the whole strip, issued up-front in
                # consumption order on the SYNC queue only: a WAR-delayed
                # prefetch there can only delay later prefetches, never the
                # scalar queue's EXPs/copies or the gpsimd queue's work.
                # x/tables for strips >0 were hoisted into the previous
                # strip's attention phase.  Strip 0's leading tiles are
                # split into pieces across sync+scalar (those DMAs never
                # wait) for a fast start.
                wq_tiles = []
                if c > 0:
                    xt_h = nxt.pop("x")
                    tabs = nxt.pop("tabs")
                else:
                    xt_h = []
                for jt in range(NJT):
                    w_ = wqp.tile([128, NET, 128], bf16, tag="wq", name="wq_sb")
                    if c == 0 and jt < 2:
                        # interleave the first weight tiles and x halves in
                        # small pieces across both HW queues; geometrically
                        # growing pieces so the very first matmuls can start
                        # after ~0.4 MB has landed
                        xh = xtp.tile([128, NET // 2, CW], bf16, tag="xt",
                                      name="xh")
                        cuts = (0, 4, 8, 16, 32) if jt == 0 else (0, 16, 32)
                        for pi in range(len(cuts) - 1):
                            w0, w1 = cuts[pi], cuts[pi + 1]
                            e0, e1 = w0 // 2, w1 // 2
                            nc.sync.dma_start(out=w_[:, w0:w1, :],
                                              in_=wq_d[jt, :, w0:w1, :])
                            nc.scalar.dma_start(
                                out=xh[:, e0:e1, :],
                                in_=xt_d[c, :, jt * (NET // 2) + e0:
                                         jt * (NET // 2) + e1, :])
                        xt_h.append(xh)
                    else:
                        nc.sync.dma_start(out=w_, in_=wq_d[jt])
                    wq_tiles.append(w_)
                    if c == 0 and jt == 3:
                        tabs = load_tables(c)
                if c == 0:
                    nc.sync.dma_start(out=mk_sb, in_=mk_d[:])
                cq_sb, sq_sb, ck_sb, sk_sb = tabs

                # ---- fused QKV projection + RoPE + v transpose, per kv group ----
                q_sb = qp.tile([128, HPC, CW], bf16, tag="q")
                for g in range(GPC):
                    stage = []
                    for sub in range(6):     # 4 q tiles, 1 k tile, 1 v tile
                        jt = 6 * g + sub
                        wq_sb = wq_tiles[jt]
                        ps = pmm.tile([128, CW], f32, tag="mm")
                        for et in range(NET):
                            nc.tensor.matmul(
                                ps,
                                lhsT=wq_sb[:, et, :],
                                rhs=xt_h[et // (NET // 2)][:, et % (NET // 2), :],
                                start=(et == 0),
                                stop=(et == NET - 1),
                            )
                        st = stp.tile([128, CW], f32, tag="st")
                        nc.scalar.copy(st, ps)
                        stage.append(st)
                    # RoPE on 4 q tiles + 1 k tile; features are
                    # de-interleaved (pair halves in partitions 0-63/64-127)
                    # so the pair-swap is two contiguous half-partition DMAs
                    for sub in range(QPK + 1):
                        stq = stage[sub]
                        is_q = sub < QPK
                        c_tab = cq_sb if is_q else ck_sb
                        s_tab = sq_sb if is_q else sk_sb
                        sw = rtp.tile([128, CW], f32, tag="sw")
                        nc.gpsimd.dma_start(out=sw[0:64, :], in_=stq[64:128, :])
                        nc.gpsimd.dma_start(out=sw[64:128, :], in_=stq[0:64, :])
                        xs = rtp.tile([128, CW], f32, tag="xs")
                        nc.vector.tensor_mul(xs, sw, s_tab)
                        nc.vector.tensor_mul(stq, stq, c_tab)
                        if is_q:
                            nc.vector.tensor_add(q_sb[:, QPK * g + sub, :],
                                                 stq, xs)
                        else:
                            nc.vector.tensor_add(k_sb[g][:, csl], stq, xs)
                    # v transpose on the PE (never touches a DMA queue); uses
                    # the qk PSUM pool, which is idle during the projection
                    # phase, so it never contends with the matmul chains
                    stv = stage[5]
                    for u in range(CW // 128):
                        tp_ = pqk.tile([128, CW], f32, tag="qk")
                        nc.tensor.transpose(
                            tp_[:, :128], stv[:, u * 128:(u + 1) * 128], ident
                        )
                        nc.scalar.copy(
                            v_sb[g][:, (CW // 128) * c + u, :], tp_[:, :128]
                        )

                # ---- attention for q strip c (flash-style, no max) ----
                njt2 = (CW // 128) * (c + 1)     # causal: k tiles 0..4c+3
                ctx_sb = ctxp.tile([128, HPC, CW], bf16, tag="ctx")
                if c > 0:
                    wo_q = [load_wo(0), load_wo(1)]
                for g in range(GPC):
                    for hq in range(QPK):
                        h = QPK * g + hq
                        if c > 0:
                            # software pipeline: strip c-1's output projection
                            # block (ec = h) fills PE while ACT/DVE run
                            # softmax; block weights prefetch two head-blocks
                            # ahead
                            if h < ECN - 2:
                                wo_q.append(load_wo(h + 2))
                            elif c == SC - 1:
                                # pre-load the drain's first blocks while the
                                # queues are quiet (no next-strip prefetch)
                                wo_q.append(load_wo(h - ECN + 2, drain=True))
                            emit_wo_block(c - 1, h, prev_ctx, wo_q.pop(0))
                        ctx_ps = pacc.tile([128, CW], f32, tag="acc")
                        sums_ps = pacc.tile([128, CW], f32, tag="acc")
                        at_acc = None
                        nsg = (njt2 + SB - 1) // SB   # ones-matmul groups
                        for j2 in range(njt2):
                            # diagonal k-tiles: trim the fully-masked columns
                            # from the QK matmul, exp and AV; zero-fill that
                            # part of the attn tile so the row-sums stay
                            # full-width
                            diag = j2 >= njt2 - (CW // 128)
                            o = 128 * (j2 - (njt2 - (CW // 128))) if diag else 0
                            nw = CW - o
                            qk = pqk.tile([128, CW], f32, tag="qk")
                            nc.tensor.matmul(
                                qk[:, :nw],
                                lhsT=k_sb[g][:, j2 * 128:(j2 + 1) * 128],
                                rhs=q_sb[:, h, o:],
                                start=True, stop=True,
                            )
                            at = atp.tile([128, CW], bf16, tag="at")
                            if o:
                                nc.gpsimd.memset(at[:, :o], 0.0)
                            nc.scalar.activation(
                                at[:, o:], qk[:, :nw],
                                mybir.ActivationFunctionType.Exp
                            )
                            if diag:
                                nc.vector.tensor_mul(
                                    at[:, o:o + 128], at[:, o:o + 128],
                                    mk_sb,
                                )
                            first, last = j2 == 0, j2 == njt2 - 1
                            if o:
                                nc.tensor.matmul(
                                    ctx_ps[:, o:], lhsT=v_sb[g][:, j2, :],
                                    rhs=at[:, o:], start=False, stop=last,
                                )
                            else:
                                nc.tensor.matmul(
                                    ctx_ps, lhsT=v_sb[g][:, j2, :], rhs=at,
                                    start=first, stop=last,
                                )
                            # batch the row-sum matmul over groups of SB attn
                            # tiles: accumulate on DVE (bf16), one ones-matmul
                            # per group instead of per tile
                            ph = j2 % SB
                            if ph == 0:
                                at_prev = at
                            elif ph == 1:
                                at_acc = atp.tile([128, CW], bf16, tag="ata",
                                                  name="at_acc", bufs=3)
                                nc.vector.tensor_add(at_acc, at_prev, at)
                            else:
                                nc.vector.tensor_add(at_acc, at_acc, at)
                            if ph == SB - 1 or j2 == njt2 - 1:
                                grp = j2 // SB
                                src = at_acc if ph else at_prev
                                nc.tensor.matmul(
                                    sums_ps, lhsT=ones_sb, rhs=src,
                                    start=(grp == 0), stop=(grp == nsg - 1),
                                )
                        rc = rcp.tile([128, CW], f32, tag="rc")
                        nc.vector.reciprocal_approx_fast(out=rc, in_=sums_ps)
                        nc.vector.tensor_mul(ctx_sb[:, h, :], ctx_ps, rc)

                        # hoist the next strip's x/table loads into this
                        # attention phase, one piece per head so the wo
                        # loads interleaved above are never delayed by more
                        # than ~1 MB of queue occupancy
                        if c < SC - 1:
                            if h == 0:
                                nxt["x"] = [
                                    xtp.tile([128, NET // 2, CW], bf16,
                                             tag="xt", name="xh")
                                    for _ in range(2)
                                ]
                            if h < 4:
                                s0, s1 = (h % 2) * (NET // 4), \
                                    (h % 2 + 1) * (NET // 4)
                                nc.sync.dma_start(
                                    out=nxt["x"][h // 2][:, s0:s1, :],
                                    in_=xt_d[c + 1, :,
                                             (h // 2) * (NET // 2) + s0:
                                             (h // 2) * (NET // 2) + s1, :])
                            elif h == 4:
                                nxt["tabs"] = load_tables(c + 1)

                prev_ctx = ctx_sb

            # drain: output projection for the final strip (blocks 0 and 1
            # were pre-loaded during the last attention phase)
            for ec in range(ECN):
                if ec < ECN - 2:
                    wo_q.append(load_wo(ec + 2, drain=True))
                emit_wo_block(SC - 1, ec, prev_ctx, wo_q.pop(0), drain=True)
    nc.finalize()
    return nc


def _rope_tables(scale):
    """De-interleaved rope tables [128, S]: partition p<64 holds pair-lane 1
    (original feature 2p), p>=64 holds pair-lane 2 (feature 2(p-64)+1).
    The S table multiplies the half-SWAPPED tensor (sw[p<64] = x2):
      out_top = x1*cos + sw_top * (-sin) = x1 cos - x2 sin
      out_bot = x2*cos + sw_bot * (+sin) = x2 cos + x1 sin
    """
    inv = 1.0 / (ROPE_BASE ** (np.arange(0, D, 2, dtype=np.float64) / D))
    ang = np.arange(S, dtype=np.float64)[None, :] * inv[:, None]    # [D/2, S]
    C = np.empty((D, S), np.float32)
    Sx = np.empty((D, S), np.float32)
    C[:64] = np.cos(ang)
    C[64:] = np.cos(ang)
    Sx[:64] = -np.sin(ang)
    Sx[64:] = np.sin(ang)
    return (C * scale).astype(np.float32), (Sx * scale).astype(np.float32)


_DEPERM = np.concatenate([np.arange(0, D, 2), np.arange(1, D, 2)])


def _host_inputs(x, wqkv, wo):
    """Shard + retile inputs for the 8 cores. Core c = 4*db + t."""
    cq, sq = _rope_tables(D ** -0.5)
    ck, sk = _rope_tables(1.0)

    # causal mask for the 128x128 diagonal block, scores^T layout [k, q]
    jj = np.arange(128)[:, None]
    ii = np.arange(128)[None, :]
    mk = (jj <= ii).astype(np_bf16)

    # de-interleave the q/k head feature rows of wqkv (see _rope_tables);
    # v rows keep their original order
    wq_p = wqkv.reshape(KVH, QPK + 2, D, E).copy()
    wq_p[:, :QPK + 1] = wq_p[:, :QPK + 1, _DEPERM]
    wq_p = wq_p.reshape(E + 2 * KVH * D, E)

    xts = []
    for db in range(DP):
        xT = np.ascontiguousarray(x[db].T)                 # [E, S]
        t = xT.reshape(NET, 128, SC, CW).transpose(2, 1, 0, 3)
        xts.append(np.ascontiguousarray(t.astype(np_bf16)))

    wqs, wos = [], []
    rows = (E + 2 * KVH * D) // TP
    for t in range(TP):
        wT = np.ascontiguousarray(wq_p[rows * t:rows * (t + 1)].T)   # [E, 1536]
        wq_t = wT.reshape(NET, 128, NJT, 128).transpose(2, 1, 0, 3)
        wqs.append(np.ascontiguousarray(wq_t.astype(np_bf16)))
        woT = np.ascontiguousarray(wo[:, 1024 * t:1024 * (t + 1)].T)  # [1024, E]
        wo_t = woT.reshape(FT, 128, ECN, CW).transpose(2, 1, 0, 3)
        wos.append(np.ascontiguousarray(wo_t.astype(np_bf16)))

    in_maps = []
    for c in range(NCORES):
        db, t = divmod(c, TP)
        in_maps.append({
            "xt": xts[db], "wq": wqs[t], "wo": wos[t],
            "cq": cq, "sq": sq, "ck": ck, "sk": sk,
            "mk": mk,
        })
    return in_maps


def kernel(x, wqkv, wo):
    x = np.asarray(x, np.float32)
    wqkv = np.asarray(wqkv, np.float32)
    wo = np.asarray(wo, np.float32)

    if "nc" not in _built:
        _built["nc"] = _build_nc()
    nc = _built["nc"]

    if os.environ.get("BASS_TRACE") and "warm" not in _built:
        # axon's NTFF profile start returns -1 until the PJRT client has
        # fully initialized (first execute), so force one tiny op first.
        try:
            import jax
            jax.jit(lambda a: a + 1)(np.zeros(1, np.float32))
        except Exception:
            pass
        _built["warm"] = True

    in_maps = _host_inputs(x, wqkv, wo)
    res = run_bass_kernel_spmd(nc, in_maps, core_ids=list(range(NCORES)))
    globals()["_last_results"] = res

    out = np.zeros((B, S, E), np.float32)
    for c in range(NCORES):
        db = c // TP
        o = res.results[c]["out"]                       # [16, 8, 128, 512] bf16
        out[db] += o.astype(np.float32).transpose(0, 2, 1, 3).reshape(S, E)
    return out


# revision 41
# speedup vs baseline: 1.0228x; 1.0183x over previous
"""Bass/Trainium2 kernel for GQA transformer block (nn_GQA_84353157694016).

Reference computation (B=2, S=2048, E=4096, H=32 q-heads, KVH=8 kv-heads, D=128):
    qkv = x @ wqkv.T                  -> split into q/k/v per GQA group
    q,k = rope_interleaved(q), rope_interleaved(k)
    out = softmax(causal(q k^T / sqrt(D))) @ v @ wo.T

Sharding (8 cores): data-parallel over batch (2 groups of 4 cores) x
tensor-parallel over heads (4 cores: 2 kv groups / 8 q heads each).  wo is
sharded on its input dim; the partial outputs are summed on the host
(the unshard step of the reduce).

Layout strategy: everything on-chip is computed in "transposed" (feature x
sequence) orientation so the TensorE contraction dim always lands on
partitions with zero on-chip transposes (v is transposed via the DMA XBAR).
Softmax is computed without max-subtraction (scores are bounded ~ +-10 for
this problem size/scale), with the row-sum obtained by an extra ones-matmul
in the same PSUM-accumulation pass as attn @ v.

The q/k feature dims are de-interleaved host-side (wqkv row permutation) so
RoPE's pair-swap becomes two half-partition adds on the vector engine
instead of strided SBUF-to-SBUF DMAs.  QK^T dot products are invariant to
the (shared) feature permutation; v rows are left in original order.
"""

import os
import sys

import numpy as np
import ml_dtypes

for _p in ("/opt/trn_rl_repo",):
    if _p not in sys.path and os.path.isdir(_p):
        sys.path.append(_p)

import concourse.bass as bass
import concourse.tile as tile
from concourse import bacc, mybir
from concourse.bass_utils import run_bass_kernel_spmd
from concourse.masks import make_identity


def _install_ntff_hook():
    """bass_utils' trace path imports antenv.axon_hooks, which the agent image
    lacks; synthesize it (backed by trn_boot's ctypes NTFF driver) so
    trace=True / BASS_TRACE=1 works instead of crashing."""
    try:
        import antenv.axon_hooks  # noqa: F401
        return
    except ImportError:
        pass
    try:
        import types
        import antenv
        mod = types.ModuleType("antenv.axon_hooks")
        mod._hook = None
        mod.set_axon_ntff_profile_hook = lambda h: setattr(mod, "_hook", h)
        mod.get_axon_ntff_profile_hook = lambda: mod._hook
        sys.modules["antenv.axon_hooks"] = mod
        antenv.axon_hooks = mod
        from trn_agent_boot.trn_boot import _ntff_profile_via_ctypes
        so = "/opt/axon/libaxon_pjrt.so"
        if os.path.exists(so):
            mod._hook = _ntff_profile_via_ctypes(so)
    except Exception:
        pass


_install_ntff_hook()

# problem constants
B, S, E = 2, 2048, 4096
H, KVH, D = 32, 8, 128
QPK = H // KVH                    # 4 q heads per kv group
ROPE_BASE = 10000.0

NCORES = 8
TP = 4                            # tensor-parallel width (heads)
DP = 2                            # data-parallel width (batch)

SC = 4                            # number of s-chunks == q strips
CW = S // SC                      # 512 chunk width
NJT = (E + 2 * KVH * D) // TP // 128   # 12 qkv row-tiles per core
NET = E // 128                    # 32 contraction tiles for qkv proj
GPC = KVH // TP                   # 2 kv groups per core
HPC = H // TP                     # 8 q heads per core
FT = HPC * D // 128               # 8 local ctx feature tiles
ECN = E // CW                     # 8 output e-chunks
SB = 8                            # row-sum ones-matmul batch size

f32 = mybir.dt.float32
bf16 = mybir.dt.bfloat16
np_bf16 = ml_dtypes.bfloat16

_built = {}


def _build_nc():
    nc = bacc.Bacc("TRN2", target_bir_lowering=False)

    xt_d = nc.dram_tensor("xt", [SC, 128, NET, CW], bf16, kind="ExternalInput")
    wq_d = nc.dram_tensor("wq", [NJT, 128, NET, 128], bf16, kind="ExternalInput")
    wo_d = nc.dram_tensor("wo", [ECN, 128, FT, CW], bf16, kind="ExternalInput")
    cq_d = nc.dram_tensor("cq", [128, S], f32, kind="ExternalInput")
    sq_d = nc.dram_tensor("sq", [128, S], f32, kind="ExternalInput")
    ck_d = nc.dram_tensor("ck", [128, S], f32, kind="ExternalInput")
    sk_d = nc.dram_tensor("sk", [128, S], f32, kind="ExternalInput")
    mk_d = nc.dram_tensor("mk", [128, 128], bf16, kind="ExternalInput")
    out_d = nc.dram_tensor("out", [S // 128, ECN, 128, CW], bf16,
                           kind="ExternalOutput")

    with tile.TileContext(nc) as tc:
        with (
            tc.tile_pool(name="const", bufs=1) as constp,
            tc.tile_pool(name="tab", bufs=1) as tabp,
            tc.tile_pool(name="xt", bufs=3) as xtp,
            tc.tile_pool(name="wq", bufs=6) as wqp,
            tc.tile_pool(name="st", bufs=7) as stp,
            tc.tile_pool(name="rt", bufs=2) as rtp,
            tc.tile_pool(name="q", bufs=1) as qp,
            tc.tile_pool(name="kv", bufs=1) as kvp,
            tc.tile_pool(name="at", bufs=8) as atp,
            tc.tile_pool(name="ctx", bufs=2) as ctxp,
            tc.tile_pool(name="wop", bufs=3) as wop,
            tc.tile_pool(name="ob", bufs=3) as obp,
            tc.tile_pool(name="rc", bufs=1) as rcp,
            tc.tile_pool(name="pmm", bufs=2, space="PSUM") as pmm,
            tc.tile_pool(name="pqk", bufs=3, space="PSUM") as pqk,
            tc.tile_pool(name="pacc", bufs=3, space="PSUM") as pacc,
        ):
            def load_wo(ec, drain=False):
                """Start the wo tile load for output chunk ec.  Halves split
                across sync (clear of weight prefetch during the attention
                phase) and gpsimd; never scalar, whose FIFO carries the
                critical EXPs -- except in the drain, where there are no
                EXPs left and gpsimd (SW DGE) is the laggard."""
                wo_sb = wop.tile([128, FT, CW], bf16, tag="wo", name="wo_sb")
                eng2 = nc.scalar if drain else nc.gpsimd
                nc.sync.dma_start(out=wo_sb[:, :FT // 2, :],
                                  in_=wo_d[ec, :, :FT // 2, :])
                eng2.dma_start(out=wo_sb[:, FT // 2:, :],
                               in_=wo_d[ec, :, FT // 2:, :])
                return wo_sb

            def emit_wo_block(cs, ec, ctx_tiles, wo_sb, drain=False):
                """Output-projection block: out[strip cs, ec] += ctx @ woT.
                PSUM comes from the projection-chain pool (idle during the
                attention phase) so the wo pipeline never write-after-read
                stalls on the previous head's softmax tail in pacc."""
                eng2 = nc.scalar if drain else nc.gpsimd
                for sti in range(CW // 128):
                    ps = pmm.tile([128, CW], f32, tag="mm", name="wo_ps")
                    for ft in range(FT):
                        nc.tensor.matmul(
                            ps,
                            lhsT=ctx_tiles[:, ft, sti * 128:(sti + 1) * 128],
                            rhs=wo_sb[:, ft, :],
                            start=(ft == 0),
                            stop=(ft == FT - 1),
                        )
                    ob = obp.tile([128, CW], bf16, tag="ob", name="ob")
                    nc.vector.tensor_copy(ob, ps)
                    eng2.dma_start(
                        out=out_d[(CW // 128) * cs + sti, ec], in_=ob
                    )

            # constants
            ident = constp.tile([128, 128], f32, tag="ident")
            make_identity(nc, ident)
            ones_sb = constp.tile([128, 128], bf16, tag="ones")
            nc.vector.memset(ones_sb, 1.0)
            mk_sb = constp.tile([128, 128], bf16, tag="mk")

            # persistent k (transposed) and v (natural) per kv group, bf16
            k_sb = [kvp.tile([128, S], bf16, tag=f"k{g}", name=f"k{g}")
                    for g in range(GPC)]
            v_sb = [kvp.tile([128, S // 128, 128], bf16, tag=f"v{g}", name=f"v{g}")
                    for g in range(GPC)]

            def load_tables(c):
                """Rope table slices for strip c (needed only at RoPE time)."""
                csl = slice(c * CW, (c + 1) * CW)
                cq_sb = tabp.tile([128, CW], f32, tag="cq")
                sq_sb = tabp.tile([128, CW], f32, tag="sq")
                ck_sb = tabp.tile([128, CW], f32, tag="ck")
                sk_sb = tabp.tile([128, CW], f32, tag="sk")
                nc.sync.dma_start(out=cq_sb, in_=cq_d[:, csl])
                nc.sync.dma_start(out=sq_sb, in_=sq_d[:, csl])
                nc.sync.dma_start(out=ck_sb, in_=ck_d[:, csl])
                nc.sync.dma_start(out=sk_sb, in_=sk_d[:, csl])
                return cq_sb, sq_sb, ck_sb, sk_sb

            nxt = {}
            for c in range(SC):
                csl = slice(c * CW, (c + 1) * CW)
                # Weight loads for the whole strip, issued up-front in
                # consumption order on the SYNC queue only: a WAR-delayed
                # prefetch there can only delay later prefetches, never the
                # scalar queue's EXPs/copies or the gpsimd queue's work.
                # x/tables for strips >0 were hoisted into the previous
                # strip's attention phase.  Strip 0's leading tiles are
                # split into pieces across sync+scalar (those DMAs never
                # wait) for a fast start.
                wq_tiles = []
                if c > 0:
                    xt_h = nxt.pop("x")
                    tabs = nxt.pop("tabs")
                else:
                    xt_h = []
                for jt in range(NJT):
                    w_ = wqp.tile([128, NET, 128], bf16, tag="wq", name="wq_sb")
                    if c == 0 and jt < 2:
                        # interleave the first weight tiles and x halves in
                        # small pieces across both HW queues; geometrically
                        # growing pieces so the very first matmuls can start
                        # after ~0.4 MB has landed
                        xh = xtp.tile([128, NET // 2, CW], bf16, tag="xt",
                                      name="xh")
                        cuts = (0, 4, 8, 16, 32) if jt == 0 else (0, 16, 32)
                        for pi in range(len(cuts) - 1):
                            w0, w1 = cuts[pi], cuts[pi + 1]
                            e0, e1 = w0 // 2, w1 // 2
                            nc.sync.dma_start(out=w_[:, w0:w1, :],
                                              in_=wq_d[jt, :, w0:w1, :])
                            nc.scalar.dma_start(
                                out=xh[:, e0:e1, :],
                                in_=xt_d[c, :, jt * (NET // 2) + e0:
                                         jt * (NET // 2) + e1, :])
                        xt_h.append(xh)
                    else:
                        nc.sync.dma_start(out=w_, in_=wq_d[jt])
                    wq_tiles.append(w_)
                    if c == 0 and jt == 3:
                        tabs = load_tables(c)
                if c == 0:
                    nc.sync.dma_start(out=mk_sb, in_=mk_d[:])
                cq_sb, sq_sb, ck_sb, sk_sb = tabs

                # ---- fused QKV projection + RoPE + v transpose, per kv group ----
                q_sb = qp.tile([128, HPC, CW], bf16, tag="q")
                for g in range(GPC):
                    stage = []
                    for sub in range(6):     # 4 q tiles, 1 k tile, 1 v tile
                        jt = 6 * g + sub
                        wq_sb = wq_tiles[jt]
                        ps = pmm.tile([128, CW], f32, tag="mm")
                        for et in range(NET):
                            nc.tensor.matmul(
                                ps,
                                lhsT=wq_sb[:, et, :],
                                rhs=xt_h[et // (NET // 2)][:, et % (NET // 2), :],
                                start=(et == 0),
                                stop=(et == NET - 1),
                            )
                        st = stp.tile([128, CW], f32, tag="st")
                        nc.scalar.copy(st, ps)
                        stage.append(st)
                    # RoPE on 4 q tiles + 1 k tile; features are
                    # de-interleaved (pair halves in partitions 0-63/64-127)
                    # so the pair-swap is two contiguous half-partition DMAs
                    for sub in range(QPK + 1):
                        stq = stage[sub]
                        is_q = sub < QPK
                        c_tab = cq_sb if is_q else ck_sb
                        s_tab = sq_sb if is_q else sk_sb
                        sw = rtp.tile([128, CW], f32, tag="sw")
                        nc.gpsimd.dma_start(out=sw[0:64, :], in_=stq[64:128, :])
                        nc.gpsimd.dma_start(out=sw[64:128, :], in_=stq[0:64, :])
                        xs = rtp.tile([128, CW], f32, tag="xs")
                        nc.vector.tensor_mul(xs, sw, s_tab)
                        nc.vector.tensor_mul(stq, stq, c_tab)
                        if is_q:
                            nc.vector.tensor_add(q_sb[:, QPK * g + sub, :],
                                                 stq, xs)
                        else:
                            nc.vector.tensor_add(k_sb[g][:, csl], stq, xs)
                    # v transpose on the PE (never touches a DMA queue); uses
                    # the qk PSUM pool, which is idle during the projection
                    # phase, so it never contends with the matmul chains
                    stv = stage[5]
                    for u in range(CW // 128):
                        tp_ = pqk.tile([128, CW], f32, tag="qk")
                        nc.tensor.transpose(
                            tp_[:, :128], stv[:, u * 128:(u + 1) * 128], ident
                        )
                        nc.scalar.copy(
                            v_sb[g][:, (CW // 128) * c + u, :], tp_[:, :128]
                        )

                # ---- attention for q strip c (flash-style, no max) ----
                njt2 = (CW // 128) * (c + 1)     # causal: k tiles 0..4c+3
                ctx_sb = ctxp.tile([128, HPC, CW], bf16, tag="ctx")
                if c > 0:
                    wo_q = [load_wo(0), load_wo(1)]
                for g in range(GPC):
                    for hq in range(QPK):
                        h = QPK * g + hq
                        if c > 0:
                            # software pipeline: strip c-1's output projection
                            # block (ec = h) fills PE while ACT/DVE run
                            # softmax; block weights prefetch two head-blocks
                            # ahead
                            if h < ECN - 2:
                                wo_q.append(load_wo(h + 2))
                            elif c == SC - 1:
                                # pre-load the drain's first blocks while the
                                # queues are quiet (no next-strip prefetch)
                                wo_q.append(load_wo(h - ECN + 2, drain=True))
                            emit_wo_block(c - 1, h, prev_ctx, wo_q.pop(0))
                        ctx_ps = pacc.tile([128, CW], f32, tag="acc")
                        sums_ps = pacc.tile([128, CW], f32, tag="acc")
                        at_acc = None
                        nsg = (njt2 + SB - 1) // SB   # ones-matmul groups
                        for j2 in range(njt2):
                            # diagonal k-tiles: trim the fully-masked columns
                            # from the QK matmul, exp and AV; zero-fill that
                            # part of the attn tile so the row-sums stay
                            # full-width
                            diag = j2 >= njt2 - (CW // 128)
                            o = 128 * (j2 - (njt2 - (CW // 128))) if diag else 0
                            nw = CW - o
                            qk = pqk.tile([128, CW], f32, tag="qk")
                            nc.tensor.matmul(
                                qk[:, :nw],
                                lhsT=k_sb[g][:, j2 * 128:(j2 + 1) * 128],
                                rhs=q_sb[:, h, o:],
                                start=True, stop=True,
                            )
                            at = atp.tile([128, CW], bf16, tag="at")
                            if o:
                                nc.gpsimd.memset(at[:, :o], 0.0)
                            nc.scalar.activation(
                                at[:, o:], qk[:, :nw],
                                mybir.ActivationFunctionType.Exp
                            )
                            if diag:
                                nc.vector.tensor_mul(
                                    at[:, o:o + 128], at[:, o:o + 128],
                                    mk_sb,
                                )
                            first, last = j2 == 0, j2 == njt2 - 1
                            if o:
                                nc.tensor.matmul(
                                    ctx_ps[:, o:], lhsT=v_sb[g][:, j2, :],
                                    rhs=at[:, o:], start=False, stop=last,
                                )
                            else:
                                nc.tensor.matmul(
                                    ctx_ps, lhsT=v_sb[g][:, j2, :], rhs=at,
                                    start=first, stop=last,
                                )
                            # batch the row-sum matmul over groups of SB attn
                            # tiles: accumulate on DVE (bf16), one ones-matmul
                            # per group instead of per tile
                            ph = j2 % SB
                            if ph == 0:
                                at_prev = at
                            elif ph == 1:
                                at_acc = atp.tile([128, CW], bf16, tag="ata",
                                                  name="at_acc", bufs=3)
                                nc.vector.tensor_add(at_acc, at_prev, at)
                            else:
                                nc.vector.tensor_add(at_acc, at_acc, at)
                            if ph == SB - 1 or j2 == njt2 - 1:
                                grp = j2 // SB
                                src = at_acc if ph else at_prev
                                nc.tensor.matmul(
                                    sums_ps, lhsT=ones_sb, rhs=src,
                                    start=(grp == 0), stop=(grp == nsg - 1),
                                )
                        rc = rcp.tile([128, CW], f32, tag="rc")
                        nc.vector.reciprocal_approx_fast(out=rc, in_=sums_ps)
                        nc.vector.tensor_mul(ctx_sb[:, h, :], ctx_ps, rc)

                        # hoist the next strip's x/table loads into this
                        # attention phase, one piece per head so the wo
                        # loads interleaved above are never delayed by more
                        # than ~1 MB of queue occupancy
                        if c < SC - 1:
                            if h == 0:
                                nxt["x"] = [
                                    xtp.tile([128, NET // 2, CW], bf16,
                                             tag="xt", name="xh")
                                    for _ in range(2)
                                ]
                            if h < 4:
                                s0, s1 = (h % 2) * (NET // 4), \
                                    (h % 2 + 1) * (NET // 4)
                                nc.sync.dma_start(
                                    out=nxt["x"][h // 2][:, s0:s1, :],
                                    in_=xt_d[c + 1, :,
                                             (h // 2) * (NET // 2) + s0:
                                             (h // 2) * (NET // 2) + s1, :])
                            elif h == 4:
                                nxt["tabs"] = load_tables(c + 1)

                prev_ctx = ctx_sb

            # drain: output projection for the final strip (blocks 0 and 1
            # were pre-loaded during the last attention phase)
            for ec in range(ECN):
                if ec < ECN - 2:
                    wo_q.append(load_wo(ec + 2, drain=True))
                emit_wo_block(SC - 1, ec, prev_ctx, wo_q.pop(0), drain=True)
    nc.finalize()
    return nc


def _rope_tables(scale):
    """De-interleaved rope tables [128, S]: partition p<64 holds pair-lane 1
    (original feature 2p), p>=64 holds pair-lane 2 (feature 2(p-64)+1).
    The S table multiplies the half-SWAPPED tensor (sw[p<64] = x2):
      out_top = x1*cos + sw_top * (-sin) = x1 cos - x2 sin
      out_bot = x2*cos + sw_bot * (+sin) = x2 cos + x1 sin
    """
    inv = 1.0 / (ROPE_BASE ** (np.arange(0, D, 2, dtype=np.float64) / D))
    ang = np.arange(S, dtype=np.float64)[None, :] * inv[:, None]    # [D/2, S]
    C = np.empty((D, S), np.float32)
    Sx = np.empty((D, S), np.float32)
    C[:64] = np.cos(ang)
    C[64:] = np.cos(ang)
    Sx[:64] = -np.sin(ang)
    Sx[64:] = np.sin(ang)
    return (C * scale).astype(np.float32), (Sx * scale).astype(np.float32)


_DEPERM = np.concatenate([np.arange(0, D, 2), np.arange(1, D, 2)])


def _host_inputs(x, wqkv, wo):
    """Shard + retile inputs for the 8 cores. Core c = 4*db + t."""
    cq, sq = _rope_tables(D ** -0.5)
    ck, sk = _rope_tables(1.0)

    # causal mask for the 128x128 diagonal block, scores^T layout [k, q]
    jj = np.arange(128)[:, None]
    ii = np.arange(128)[None, :]
    mk = (jj <= ii).astype(np_bf16)

    # de-interleave the q/k head feature rows of wqkv (see _rope_tables);
    # v rows keep their original order
    wq_p = wqkv.reshape(KVH, QPK + 2, D, E).copy()
    wq_p[:, :QPK + 1] = wq_p[:, :QPK + 1, _DEPERM]
    wq_p = wq_p.reshape(E + 2 * KVH * D, E)

    xts = []
    for db in range(DP):
        xT = np.ascontiguousarray(x[db].T)                 # [E, S]
        t = xT.reshape(NET, 128, SC, CW).transpose(2, 1, 0, 3)
        xts.append(np.ascontiguousarray(t.astype(np_bf16)))

    wqs, wos = [], []
    rows = (E + 2 * KVH * D) // TP
    for t in range(TP):
        wT = np.ascontiguousarray(wq_p[rows * t:rows * (t + 1)].T)   # [E, 1536]
        wq_t = wT.reshape(NET, 128, NJT, 128).transpose(2, 1, 0, 3)
        wqs.append(np.ascontiguousarray(wq_t.astype(np_bf16)))
        woT = np.ascontiguousarray(wo[:, 1024 * t:1024 * (t + 1)].T)  # [1024, E]
        wo_t = woT.reshape(FT, 128, ECN, CW).transpose(2, 1, 0, 3)
        wos.append(np.ascontiguousarray(wo_t.astype(np_bf16)))

    in_maps = []
    for c in range(NCORES):
        db, t = divmod(c, TP)
        in_maps.append({
            "xt": xts[db], "wq": wqs[t], "wo": wos[t],
            "cq": cq, "sq": sq, "ck": ck, "sk": sk,
            "mk": mk,
        })
    return in_maps


def kernel(x, wqkv, wo):
    x = np.asarray(x, np.float32)
    wqkv = np.asarray(wqkv, np.float32)
    wo = np.asarray(wo, np.float32)

    if "nc" not in _built:
        _built["nc"] = _build_nc()
    nc = _built["nc"]

    if os.environ.get("BASS_TRACE") and "warm" not in _built:
        # axon's NTFF profile start returns -1 until the PJRT client has
        # fully initialized (first execute), so force one tiny op first.
        try:
            import jax
            jax.jit(lambda a: a + 1)(np.zeros(1, np.float32))
        except Exception:
            pass
        _built["warm"] = True

    in_maps = _host_inputs(x, wqkv, wo)
    res = run_bass_kernel_spmd(nc, in_maps, core_ids=list(range(NCORES)))
    globals()["_last_results"] = res

    out = np.zeros((B, S, E), np.float32)
    for c in range(NCORES):
        db = c // TP
        o = res.results[c]["out"]                       # [16, 8, 128, 512] bf16
        out[db] += o.astype(np.float32).transpose(0, 2, 1, 3).reshape(S, E)
    return out


# revision 42
# speedup vs baseline: 1.0229x; 1.0001x over previous
"""Bass/Trainium2 kernel for GQA transformer block (nn_GQA_84353157694016).

Reference computation (B=2, S=2048, E=4096, H=32 q-heads, KVH=8 kv-heads, D=128):
    qkv = x @ wqkv.T                  -> split into q/k/v per GQA group
    q,k = rope_interleaved(q), rope_interleaved(k)
    out = softmax(causal(q k^T / sqrt(D))) @ v @ wo.T

Sharding (8 cores): data-parallel over batch (2 groups of 4 cores) x
tensor-parallel over heads (4 cores: 2 kv groups / 8 q heads each).  wo is
sharded on its input dim; the partial outputs are summed on the host
(the unshard step of the reduce).

Layout strategy: everything on-chip is computed in "transposed" (feature x
sequence) orientation so the TensorE contraction dim always lands on
partitions with zero on-chip transposes (v is transposed via the DMA XBAR).
Softmax is computed without max-subtraction (scores are bounded ~ +-10 for
this problem size/scale), with the row-sum obtained by an extra ones-matmul
in the same PSUM-accumulation pass as attn @ v.

The q/k feature dims are de-interleaved host-side (wqkv row permutation) so
RoPE's pair-swap becomes two half-partition adds on the vector engine
instead of strided SBUF-to-SBUF DMAs.  QK^T dot products are invariant to
the (shared) feature permutation; v rows are left in original order.
"""

import os
import sys

import numpy as np
import ml_dtypes

for _p in ("/opt/trn_rl_repo",):
    if _p not in sys.path and os.path.isdir(_p):
        sys.path.append(_p)

import concourse.bass as bass
import concourse.tile as tile
from concourse import bacc, mybir
from concourse.bass_utils import run_bass_kernel_spmd
from concourse.masks import make_identity


def _install_ntff_hook():
    """bass_utils' trace path imports antenv.axon_hooks, which the agent image
    lacks; synthesize it (backed by trn_boot's ctypes NTFF driver) so
    trace=True / BASS_TRACE=1 works instead of crashing."""
    try:
        import antenv.axon_hooks  # noqa: F401
        return
    except ImportError:
        pass
    try:
        import types
        import antenv
        mod = types.ModuleType("antenv.axon_hooks")
        mod._hook = None
        mod.set_axon_ntff_profile_hook = lambda h: setattr(mod, "_hook", h)
        mod.get_axon_ntff_profile_hook = lambda: mod._hook
        sys.modules["antenv.axon_hooks"] = mod
        antenv.axon_hooks = mod
        from trn_agent_boot.trn_boot import _ntff_profile_via_ctypes
        so = "/opt/axon/libaxon_pjrt.so"
        if os.path.exists(so):
            mod._hook = _ntff_profile_via_ctypes(so)
    except Exception:
        pass


_install_ntff_hook()

# problem constants
B, S, E = 2, 2048, 4096
H, KVH, D = 32, 8, 128
QPK = H // KVH                    # 4 q heads per kv group
ROPE_BASE = 10000.0

NCORES = 8
TP = 4                            # tensor-parallel width (heads)
DP = 2                            # data-parallel width (batch)

SC = 4                            # number of s-chunks == q strips
CW = S // SC                      # 512 chunk width
NJT = (E + 2 * KVH * D) // TP // 128   # 12 qkv row-tiles per core
NET = E // 128                    # 32 contraction tiles for qkv proj
GPC = KVH // TP                   # 2 kv groups per core
HPC = H // TP                     # 8 q heads per core
FT = HPC * D // 128               # 8 local ctx feature tiles
ECN = E // CW                     # 8 output e-chunks
SB = 8                            # row-sum ones-matmul batch size

f32 = mybir.dt.float32
bf16 = mybir.dt.bfloat16
np_bf16 = ml_dtypes.bfloat16

_built = {}


def _build_nc():
    nc = bacc.Bacc("TRN2", target_bir_lowering=False)

    xt_d = nc.dram_tensor("xt", [SC, 128, NET, CW], bf16, kind="ExternalInput")
    wq_d = nc.dram_tensor("wq", [NJT, 128, NET, 128], bf16, kind="ExternalInput")
    wo_d = nc.dram_tensor("wo", [ECN, 128, FT, CW], bf16, kind="ExternalInput")
    cq_d = nc.dram_tensor("cq", [128, S], f32, kind="ExternalInput")
    sq_d = nc.dram_tensor("sq", [128, S], f32, kind="ExternalInput")
    ck_d = nc.dram_tensor("ck", [128, S], f32, kind="ExternalInput")
    sk_d = nc.dram_tensor("sk", [128, S], f32, kind="ExternalInput")
    mk_d = nc.dram_tensor("mk", [128, 128], bf16, kind="ExternalInput")
    out_d = nc.dram_tensor("out", [S // 128, ECN, 128, CW], bf16,
                           kind="ExternalOutput")

    with tile.TileContext(nc) as tc:
        with (
            tc.tile_pool(name="const", bufs=1) as constp,
            tc.tile_pool(name="tab", bufs=1) as tabp,
            tc.tile_pool(name="xt", bufs=3) as xtp,
            tc.tile_pool(name="wq", bufs=6) as wqp,
            tc.tile_pool(name="st", bufs=7) as stp,
            tc.tile_pool(name="rt", bufs=2) as rtp,
            tc.tile_pool(name="q", bufs=1) as qp,
            tc.tile_pool(name="kv", bufs=1) as kvp,
            tc.tile_pool(name="at", bufs=8) as atp,
            tc.tile_pool(name="ctx", bufs=2) as ctxp,
            tc.tile_pool(name="wop", bufs=3) as wop,
            tc.tile_pool(name="ob", bufs=3) as obp,
            tc.tile_pool(name="rc", bufs=1) as rcp,
            tc.tile_pool(name="pmm", bufs=2, space="PSUM") as pmm,
            tc.tile_pool(name="pqk", bufs=3, space="PSUM") as pqk,
            tc.tile_pool(name="pacc", bufs=3, space="PSUM") as pacc,
        ):
            def load_wo(ec, drain=False):
                """Start the wo tile load for output chunk ec.  Halves split
                across sync (clear of weight prefetch during the attention
                phase) and gpsimd; never scalar, whose FIFO carries the
                critical EXPs -- except in the drain, where there are no
                EXPs left and gpsimd (SW DGE) is the laggard."""
                wo_sb = wop.tile([128, FT, CW], bf16, tag="wo", name="wo_sb")
                eng2 = nc.scalar if drain else nc.gpsimd
                sft = FT * 3 // 4
                nc.sync.dma_start(out=wo_sb[:, :sft, :],
                                  in_=wo_d[ec, :, :sft, :])
                eng2.dma_start(out=wo_sb[:, sft:, :],
                               in_=wo_d[ec, :, sft:, :])
                return wo_sb

            def emit_wo_block(cs, ec, ctx_tiles, wo_sb, drain=False):
                """Output-projection block: out[strip cs, ec] += ctx @ woT.
                PSUM comes from the projection-chain pool (idle during the
                attention phase) so the wo pipeline never write-after-read
                stalls on the previous head's softmax tail in pacc."""
                eng2 = nc.scalar if drain else nc.gpsimd
                for sti in range(CW // 128):
                    ps = pmm.tile([128, CW], f32, tag="mm", name="wo_ps")
                    for ft in range(FT):
                        nc.tensor.matmul(
                            ps,
                            lhsT=ctx_tiles[:, ft, sti * 128:(sti + 1) * 128],
                            rhs=wo_sb[:, ft, :],
                            start=(ft == 0),
                            stop=(ft == FT - 1),
                        )
                    ob = obp.tile([128, CW], bf16, tag="ob", name="ob")
                    nc.vector.tensor_copy(ob, ps)
                    eng2.dma_start(
                        out=out_d[(CW // 128) * cs + sti, ec], in_=ob
                    )

            # constants
            ident = constp.tile([128, 128], f32, tag="ident")
            make_identity(nc, ident)
            ones_sb = constp.tile([128, 128], bf16, tag="ones")
            nc.vector.memset(ones_sb, 1.0)
            mk_sb = constp.tile([128, 128], bf16, tag="mk")

            # persistent k (transposed) and v (natural) per kv group, bf16
            k_sb = [kvp.tile([128, S], bf16, tag=f"k{g}", name=f"k{g}")
                    for g in range(GPC)]
            v_sb = [kvp.tile([128, S // 128, 128], bf16, tag=f"v{g}", name=f"v{g}")
                    for g in range(GPC)]

            def load_tables(c):
                """Rope table slices for strip c (needed only at RoPE time)."""
                csl = slice(c * CW, (c + 1) * CW)
                cq_sb = tabp.tile([128, CW], f32, tag="cq")
                sq_sb = tabp.tile([128, CW], f32, tag="sq")
                ck_sb = tabp.tile([128, CW], f32, tag="ck")
                sk_sb = tabp.tile([128, CW], f32, tag="sk")
                nc.sync.dma_start(out=cq_sb, in_=cq_d[:, csl])
                nc.sync.dma_start(out=sq_sb, in_=sq_d[:, csl])
                nc.sync.dma_start(out=ck_sb, in_=ck_d[:, csl])
                nc.sync.dma_start(out=sk_sb, in_=sk_d[:, csl])
                return cq_sb, sq_sb, ck_sb, sk_sb

            nxt = {}
            for c in range(SC):
                csl = slice(c * CW, (c + 1) * CW)
                # Weight loads for the whole strip, issued up-front in
                # consumption order on the SYNC queue only: a WAR-delayed
                # prefetch there can only delay later prefetches, never the
                # scalar queue's EXPs/copies or the gpsimd queue's work.
                # x/tables for strips >0 were hoisted into the previous
                # strip's attention phase.  Strip 0's leading tiles are
                # split into pieces across sync+scalar (those DMAs never
                # wait) for a fast start.
                wq_tiles = []
                if c > 0:
                    xt_h = nxt.pop("x")
                    tabs = nxt.pop("tabs")
                else:
                    xt_h = []
                for jt in range(NJT):
                    w_ = wqp.tile([128, NET, 128], bf16, tag="wq", name="wq_sb")
                    if c == 0 and jt < 2:
                        # interleave the first weight tiles and x halves in
                        # small pieces across both HW queues; geometrically
                        # growing pieces so the very first matmuls can start
                        # after ~0.4 MB has landed
                        xh = xtp.tile([128, NET // 2, CW], bf16, tag="xt",
                                      name="xh")
                        cuts = (0, 4, 8, 16, 32) if jt == 0 else (0, 16, 32)
                        for pi in range(len(cuts) - 1):
                            w0, w1 = cuts[pi], cuts[pi + 1]
                            e0, e1 = w0 // 2, w1 // 2
                            nc.sync.dma_start(out=w_[:, w0:w1, :],
                                              in_=wq_d[jt, :, w0:w1, :])
                            nc.scalar.dma_start(
                                out=xh[:, e0:e1, :],
                                in_=xt_d[c, :, jt * (NET // 2) + e0:
                                         jt * (NET // 2) + e1, :])
                        xt_h.append(xh)
                    else:
                        nc.sync.dma_start(out=w_, in_=wq_d[jt])
                    wq_tiles.append(w_)
                    if c == 0 and jt == 3:
                        tabs = load_tables(c)
                if c == 0:
                    nc.sync.dma_start(out=mk_sb, in_=mk_d[:])
                cq_sb, sq_sb, ck_sb, sk_sb = tabs

                # ---- fused QKV projection + RoPE + v transpose, per kv group ----
                q_sb = qp.tile([128, HPC, CW], bf16, tag="q")
                for g in range(GPC):
                    stage = []
                    for sub in range(6):     # 4 q tiles, 1 k tile, 1 v tile
                        jt = 6 * g + sub
                        wq_sb = wq_tiles[jt]
                        ps = pmm.tile([128, CW], f32, tag="mm")
                        for et in range(NET):
                            nc.tensor.matmul(
                                ps,
                                lhsT=wq_sb[:, et, :],
                                rhs=xt_h[et // (NET // 2)][:, et % (NET // 2), :],
                                start=(et == 0),
                                stop=(et == NET - 1),
                            )
                        st = stp.tile([128, CW], f32, tag="st")
                        nc.scalar.copy(st, ps)
                        stage.append(st)
                    # RoPE on 4 q tiles + 1 k tile; features are
                    # de-interleaved (pair halves in partitions 0-63/64-127)
                    # so the pair-swap is two contiguous half-partition DMAs
                    for sub in range(QPK + 1):
                        stq = stage[sub]
                        is_q = sub < QPK
                        c_tab = cq_sb if is_q else ck_sb
                        s_tab = sq_sb if is_q else sk_sb
                        sw = rtp.tile([128, CW], f32, tag="sw")
                        nc.gpsimd.dma_start(out=sw[0:64, :], in_=stq[64:128, :])
                        nc.gpsimd.dma_start(out=sw[64:128, :], in_=stq[0:64, :])
                        xs = rtp.tile([128, CW], f32, tag="xs")
                        nc.vector.tensor_mul(xs, sw, s_tab)
                        nc.vector.tensor_mul(stq, stq, c_tab)
                        if is_q:
                            nc.vector.tensor_add(q_sb[:, QPK * g + sub, :],
                                                 stq, xs)
                        else:
                            nc.vector.tensor_add(k_sb[g][:, csl], stq, xs)
                    # v transpose on the PE (never touches a DMA queue); uses
                    # the qk PSUM pool, which is idle during the projection
                    # phase, so it never contends with the matmul chains
                    stv = stage[5]
                    for u in range(CW // 128):
                        tp_ = pqk.tile([128, CW], f32, tag="qk")
                        nc.tensor.transpose(
                            tp_[:, :128], stv[:, u * 128:(u + 1) * 128], ident
                        )
                        nc.scalar.copy(
                            v_sb[g][:, (CW // 128) * c + u, :], tp_[:, :128]
                        )

                # ---- attention for q strip c (flash-style, no max) ----
                njt2 = (CW // 128) * (c + 1)     # causal: k tiles 0..4c+3
                ctx_sb = ctxp.tile([128, HPC, CW], bf16, tag="ctx")
                if c > 0:
                    wo_q = [load_wo(0), load_wo(1)]
                for g in range(GPC):
                    for hq in range(QPK):
                        h = QPK * g + hq
                        if c > 0:
                            # software pipeline: strip c-1's output projection
                            # block (ec = h) fills PE while ACT/DVE run
                            # softmax; block weights prefetch two head-blocks
                            # ahead
                            if h < ECN - 2:
                                wo_q.append(load_wo(h + 2))
                            elif c == SC - 1:
                                # pre-load the drain's first blocks while the
                                # queues are quiet (no next-strip prefetch)
                                wo_q.append(load_wo(h - ECN + 2, drain=True))
                            emit_wo_block(c - 1, h, prev_ctx, wo_q.pop(0))
                        ctx_ps = pacc.tile([128, CW], f32, tag="acc")
                        sums_ps = pacc.tile([128, CW], f32, tag="acc")
                        at_acc = None
                        nsg = (njt2 + SB - 1) // SB   # ones-matmul groups
                        for j2 in range(njt2):
                            # diagonal k-tiles: trim the fully-masked columns
                            # from the QK matmul, exp and AV; zero-fill that
                            # part of the attn tile so the row-sums stay
                            # full-width
                            diag = j2 >= njt2 - (CW // 128)
                            o = 128 * (j2 - (njt2 - (CW // 128))) if diag else 0
                            nw = CW - o
                            qk = pqk.tile([128, CW], f32, tag="qk")
                            nc.tensor.matmul(
                                qk[:, :nw],
                                lhsT=k_sb[g][:, j2 * 128:(j2 + 1) * 128],
                                rhs=q_sb[:, h, o:],
                                start=True, stop=True,
                            )
                            at = atp.tile([128, CW], bf16, tag="at")
                            if o:
                                nc.gpsimd.memset(at[:, :o], 0.0)
                            nc.scalar.activation(
                                at[:, o:], qk[:, :nw],
                                mybir.ActivationFunctionType.Exp
                            )
                            if diag:
                                nc.vector.tensor_mul(
                                    at[:, o:o + 128], at[:, o:o + 128],
                                    mk_sb,
                                )
                            first, last = j2 == 0, j2 == njt2 - 1
                            if o:
                                nc.tensor.matmul(
                                    ctx_ps[:, o:], lhsT=v_sb[g][:, j2, :],
                                    rhs=at[:, o:], start=False, stop=last,
                                )
                            else:
                                nc.tensor.matmul(
                                    ctx_ps, lhsT=v_sb[g][:, j2, :], rhs=at,
                                    start=first, stop=last,
                                )
                            # batch the row-sum matmul over groups of SB attn
                            # tiles: accumulate on DVE (bf16), one ones-matmul
                            # per group instead of per tile
                            ph = j2 % SB
                            if ph == 0:
                                at_prev = at
                            elif ph == 1:
                                at_acc = atp.tile([128, CW], bf16, tag="ata",
                                                  name="at_acc", bufs=3)
                                nc.vector.tensor_add(at_acc, at_prev, at)
                            else:
                                nc.vector.tensor_add(at_acc, at_acc, at)
                            if ph == SB - 1 or j2 == njt2 - 1:
                                grp = j2 // SB
                                src = at_acc if ph else at_prev
                                nc.tensor.matmul(
                                    sums_ps, lhsT=ones_sb, rhs=src,
                                    start=(grp == 0), stop=(grp == nsg - 1),
                                )
                        rc = rcp.tile([128, CW], f32, tag="rc")
                        nc.vector.reciprocal_approx_fast(out=rc, in_=sums_ps)
                        nc.vector.tensor_mul(ctx_sb[:, h, :], ctx_ps, rc)

                        # hoist the next strip's x/table loads into this
                        # attention phase, one piece per head so the wo
                        # loads interleaved above are never delayed by more
                        # than ~1 MB of queue occupancy
                        if c < SC - 1:
                            if h == 0:
                                nxt["x"] = [
                                    xtp.tile([128, NET // 2, CW], bf16,
                                             tag="xt", name="xh")
                                    for _ in range(2)
                                ]
                            if h < 4:
                                s0, s1 = (h % 2) * (NET // 4), \
                                    (h % 2 + 1) * (NET // 4)
                                nc.sync.dma_start(
                                    out=nxt["x"][h // 2][:, s0:s1, :],
                                    in_=xt_d[c + 1, :,
                                             (h // 2) * (NET // 2) + s0:
                                             (h // 2) * (NET // 2) + s1, :])
                            elif h == 4:
                                nxt["tabs"] = load_tables(c + 1)

                prev_ctx = ctx_sb

            # drain: output projection for the final strip (blocks 0 and 1
            # were pre-loaded during the last attention phase)
            for ec in range(ECN):
                if ec < ECN - 2:
                    wo_q.append(load_wo(ec + 2, drain=True))
                emit_wo_block(SC - 1, ec, prev_ctx, wo_q.pop(0), drain=True)
    nc.finalize()
    return nc


def _rope_tables(scale):
    """De-interleaved rope tables [128, S]: partition p<64 holds pair-lane 1
    (original feature 2p), p>=64 holds pair-lane 2 (feature 2(p-64)+1).
    The S table multiplies the half-SWAPPED tensor (sw[p<64] = x2):
      out_top = x1*cos + sw_top * (-sin) = x1 cos - x2 sin
      out_bot = x2*cos + sw_bot * (+sin) = x2 cos + x1 sin
    """
    inv = 1.0 / (ROPE_BASE ** (np.arange(0, D, 2, dtype=np.float64) / D))
    ang = np.arange(S, dtype=np.float64)[None, :] * inv[:, None]    # [D/2, S]
    C = np.empty((D, S), np.float32)
    Sx = np.empty((D, S), np.float32)
    C[:64] = np.cos(ang)
    C[64:] = np.cos(ang)
    Sx[:64] = -np.sin(ang)
    Sx[64:] = np.sin(ang)
    return (C * scale).astype(np.float32), (Sx * scale).astype(np.float32)


_DEPERM = np.concatenate([np.arange(0, D, 2), np.arange(1, D, 2)])


def _host_inputs(x, wqkv, wo):
    """Shard + retile inputs for the 8 cores. Core c = 4*db + t."""
    cq, sq = _rope_tables(D ** -0.5)
    ck, sk = _rope_tables(1.0)

    # causal mask for the 128x128 diagonal block, scores^T layout [k, q]
    jj = np.arange(128)[:, None]
    ii = np.arange(128)[None, :]
    mk = (jj <= ii).astype(np_bf16)

    # de-interleave the q/k head feature rows of wqkv (see _rope_tables);
    # v rows keep their original order
    wq_p = wqkv.reshape(KVH, QPK + 2, D, E).copy()
    wq_p[:, :QPK + 1] = wq_p[:, :QPK + 1, _DEPERM]
    wq_p = wq_p.reshape(E + 2 * KVH * D, E)

    xts = []
    for db in range(DP):
        xT = np.ascontiguousarray(x[db].T)                 # [E, S]
        t = xT.reshape(NET, 128, SC, CW).transpose(2, 1, 0, 3)
        xts.append(np.ascontiguousarray(t.astype(np_bf16)))

    wqs, wos = [], []
    rows = (E + 2 * KVH * D) // TP
    for t in range(TP):
        wT = np.ascontiguousarray(wq_p[rows * t:rows * (t + 1)].T)   # [E, 1536]
        wq_t = wT.reshape(NET, 128, NJT, 128).transpose(2, 1, 0, 3)
        wqs.append(np.ascontiguousarray(wq_t.astype(np_bf16)))
        woT = np.ascontiguousarray(wo[:, 1024 * t:1024 * (t + 1)].T)  # [1024, E]
        wo_t = woT.reshape(FT, 128, ECN, CW).transpose(2, 1, 0, 3)
        wos.append(np.ascontiguousarray(wo_t.astype(np_bf16)))

    in_maps = []
    for c in range(NCORES):
        db, t = divmod(c, TP)
        in_maps.append({
            "xt": xts[db], "wq": wqs[t], "wo": wos[t],
            "cq": cq, "sq": sq, "ck": ck, "sk": sk,
            "mk": mk,
        })
    return in_maps


def kernel(x, wqkv, wo):
    x = np.asarray(x, np.float32)
    wqkv = np.asarray(wqkv, np.float32)
    wo = np.asarray(wo, np.float32)

    if "nc" not in _built:
        _built["nc"] = _build_nc()
    nc = _built["nc"]

    if os.environ.get("BASS_TRACE") and "warm" not in _built:
        # axon's NTFF profile start returns -1 until the PJRT client has
        # fully initialized (first execute), so force one tiny op first.
        try:
            import jax
            jax.jit(lambda a: a + 1)(np.zeros(1, np.float32))
        except Exception:
            pass
        _built["warm"] = True

    in_maps = _host_inputs(x, wqkv, wo)
    res = run_bass_kernel_spmd(nc, in_maps, core_ids=list(range(NCORES)))
    globals()["_last_results"] = res

    out = np.zeros((B, S, E), np.float32)
    for c in range(NCORES):
        db = c // TP
        o = res.results[c]["out"]                       # [16, 8, 128, 512] bf16
        out[db] += o.astype(np.float32).transpose(0, 2, 1, 3).reshape(S, E)
    return out
